# revision 53
# baseline (speedup 1.0000x reference)
# Trainium2 Bass kernel for nn_EncoderBlock (dense transformer encoder block).
#
# Sharding: 8 cores, zero collectives. Core c owns batch b = c // 4 and query
# slice qs = (c % 4) * 512. Each core redundantly computes LN1/K/V for its
# whole batch (2048 tokens) and runs attention + FFN for its own 512 queries.
# The host rolls the token order per core so that the core's queries are
# tokens 0..511 of its view -- every core runs the identical SPMD program.
#
# v2 highlights over the bf16 baseline:
#  - QKV/Wo projections in fp8e4 DoubleRow (weights x16 host-prescaled,
#    evictions rescale by 1/16): half the matmul instructions.
#  - attn@v in mixed fp8 DoubleRow: v in fp8e4, softmax weights as fp8e5
#    bits; kpos chunks are consumed in pairs (contraction 256 per pass).
#  - softmax exp split across ScalarE (ACT Exp) and VectorE (Schraudolph
#    bitcast exp: bits = x*A5 + B5 -> uint8, saturation implements the mask).
#  - single ACT table set (ln/exp/identity/relu/square) -> one table load.
#  - LN stats for all four 512-token chunks are col-tiled into one [97,512]
#    PSUM tile (partitions 0/32/64/96) so the coeff chain runs once.
#  - chunk-pipelined front; attention runs in four passes (one head pair
#    each) to fit PSUM: 2 (qkv) + 4 (scores) + 2 (ov) banks.

import numpy as np

D_MODEL = 512
H = 8
DK = 64
DKP = 72          # padded per-head v row (65 used: 64 + ones column)
D_FF = 2048
B = 2
S = 2048
EPS = 1e-6

N_CORES = 8
CORES_PER_BATCH = 4
Q = 512          # queries per core
P = 128          # partitions
KD = D_MODEL // P      # 4 feature chunks
FJ = D_FF // P         # 16 ff chunks
TT = S // P            # 16 kpos chunks
TC = S // 512          # 4 token column chunks

A5 = 5.770780          # 4 / ln2      (e5m2 schraudolph scale)
B5 = 59.768            # 15*4 - 0.232 (e5m2 schraudolph offset)
WSC = 16.0             # host prescale on fp8 weights
RSC = float(1.0 / WSC)

_BUILT = None


def _emit(nc, tc, aps):
    import concourse.bass as bass
    from concourse import mybir

    f32 = mybir.dt.float32
    bf16 = mybir.dt.bfloat16
    u8 = mybir.dt.uint8
    fp8e4 = mybir.dt.float8e4
    fp8e5 = mybir.dt.float8e5
    Act = mybir.ActivationFunctionType
    Op = mybir.AluOpType
    DR = mybir.MatmulPerfMode.DoubleRow

    (xT, xq, mask, Wq, Wk, Wv, Wo, W1, W2,
     bq, bk, bv, bo, b1, b2, consts, outT) = aps

    mm = nc.tensor.matmul

    # ---------------- pools ----------------
    # LEFT stack : small, work | kv | w1, x2, h
    # RIGHT stack: wo | xq | qkvw | big(ln1) | xT | ab
    p_small = tc.alloc_tile_pool(name="p_small", bufs=1, side="left")
    p_work = tc.alloc_tile_pool(name="p_work", bufs=3, side="left")
    p_work2 = tc.alloc_tile_pool(name="p_work2", bufs=2, side="left")
    p_kv = tc.alloc_tile_pool(name="p_kv", bufs=1, side="left")
    p_wo = tc.alloc_tile_pool(name="p_wo", bufs=1, side="right")
    p_xq = tc.alloc_tile_pool(name="p_xq", bufs=1, side="right")
    p_qkvw = tc.alloc_tile_pool(name="p_qkvw", bufs=1, side="right")
    p_big = tc.alloc_tile_pool(name="p_big", bufs=1, side="right")
    p_xT = tc.alloc_tile_pool(name="p_xT", bufs=1, side="right")
    p_ab = tc.alloc_tile_pool(name="p_ab", bufs=1, side="right")

    ps_mm = tc.alloc_tile_pool(name="ps_mm", bufs=4, space="PSUM")
    ps_st = tc.alloc_tile_pool(name="ps_st", bufs=2, space="PSUM")

    # ---------------- constant / input loads ----------------
    ones_col = p_small.tile([P, 1], bf16)
    nc.vector.memset(ones_col, 1.0)
    ones_rows = p_small.tile([P, P], bf16)
    nc.vector.memset(ones_rows, 1.0)
    ones_row512 = p_small.tile([1, 512], bf16)
    nc.vector.memset(ones_row512, 1.0)
    warm_sb = p_small.tile([P, 512], bf16)
    nc.vector.memset(warm_sb, 0.0)

    # x (transposed, rotated, fp8), chunk-major so each chunk DMA is one
    # contiguous 2KB descriptor per partition; chunk 0 lands first
    xT_sb = p_xT.tile([P, TC, KD, 512], bf16)
    nc.sync.dma_start(out=xT_sb[:, 0], in_=xT[:, 0])
    nc.sync.dma_start(out=xT_sb[:, 1:TC], in_=xT[:, 1:TC])

    wq_sb = p_qkvw.tile([P, KD, D_MODEL], fp8e4)
    wk_sb = p_qkvw.tile([P, KD, D_MODEL], fp8e4)
    wv_sb = p_qkvw.tile([P, KD, D_MODEL], fp8e4)
    wo_sb = p_wo.tile([P, KD, D_MODEL], fp8e4)
    for w_sb, w in ((wq_sb, Wq), (wk_sb, Wk), (wv_sb, Wv), (wo_sb, Wo)):
        nc.sync.dma_start(out=w_sb, in_=w)
    bq_sb = p_qkvw.tile([P, KD], f32)
    bk_sb = p_qkvw.tile([P, KD], f32)
    bo_row = p_wo.tile([1, D_MODEL], f32)
    nc.sync.dma_start(out=bq_sb, in_=bq.rearrange("(j p) -> p j", p=P))
    nc.sync.dma_start(out=bk_sb, in_=bk.rearrange("(j p) -> p j", p=P))
    nc.sync.dma_start(out=bo_row, in_=bo.rearrange("(o d) -> o d", o=1))
    bv_b = p_qkvw.tile([P, D_MODEL], f32)
    nc.sync.dma_start(
        out=bv_b, in_=bv.rearrange("(o d) -> o d", o=1).to_broadcast([P, D_MODEL])
    )
    xq_sb = p_xq.tile([P, KD, Q], f32)
    nc.sync.dma_start(out=xq_sb, in_=xq)

    consts_sb = p_small.tile([1, 4], f32)
    nc.sync.dma_start(out=consts_sb, in_=consts.rearrange("(o c) -> o c", o=1))
    consts_b = p_small.tile([P, 4], f32)
    nc.gpsimd.partition_broadcast(out_ap=consts_b, in_ap=consts_sb)

    mask_i = p_small.tile([P, TT], mybir.dt.int32)
    nc.sync.dma_start(out=mask_i, in_=mask.rearrange("(t p) -> p t", p=P))
    mask_f = p_small.tile([P, TT], f32)
    nc.vector.tensor_copy(out=mask_f, in_=mask_i)
    # ACT-exp bias: mask 1 -> 0 ; mask 0 -> -1e9
    maskb8 = p_small.tile([P, TT], f32)
    nc.vector.tensor_scalar(out=maskb8, in0=mask_f, scalar1=1e9, scalar2=-1e9,
                            op0=Op.mult, op1=Op.add)
    # schraudolph bias: mask 1 -> B5 ; mask 0 -> B5 - 1e9 (saturates to 0)
    maskb5 = p_small.tile([P, TT], f32)
    nc.vector.tensor_scalar(out=maskb5, in0=mask_f, scalar1=1e9,
                            scalar2=float(B5 - 1e9), op0=Op.mult, op1=Op.add)


    # ---------------- LN1 stats (col-tiled into [97, 512]) ----------------
    xsq = p_big.tile([P, TC, KD, 512], bf16, tag="xsq")
    for t in range(TC):
        for k in range(KD):
            if t < 2:
                nc.vector.tensor_tensor(out=xsq[:, t, k, :], in0=xT_sb[:, t, k, :],
                                        in1=xT_sb[:, t, k, :], op=Op.mult)
            else:
                nc.scalar.activation(out=xsq[:, t, k, :], in_=xT_sb[:, t, k, :],
                                     func=Act.Square)

    # PE warm-up: dummy matmuls run during the input DMA wait, so the HAM
    # clock gate is at K=8/8 when the stats matmuls arrive.
    warm_ps = ps_mm.tile([P, 512], f32, tag="mm")
    for w in range(18):
        mm(warm_ps, ones_rows, warm_sb, start=(w == 0), stop=(w == 17),
           skip_group_check=True)

    s1_ps = ps_st.tile([97, 512], f32, tag="st")
    s2_ps = ps_st.tile([97, 512], f32, tag="st")
    for t in range(TC):
        for k in range(KD):
            mm(s1_ps[32 * t:32 * t + 1, :], ones_col, xT_sb[:, t, k, :],
               start=(k == 0), stop=(k == KD - 1), tile_position=(0, 32 * t))
    for t in range(TC):
        for k in range(KD):
            mm(s2_ps[32 * t:32 * t + 1, :], ones_col, xsq[:, t, k, :],
               start=(k == 0), stop=(k == KD - 1), tile_position=(0, 32 * t))

    def ln_coeffs(pool, s1_ps, s2_ps, alpha_ap, beta_ap, nrow, n_tok):
        # m = s1/n; var = (s2 - m*s1)/(n-1); a = alpha*rsqrt(var);
        # c = beta - m*a.  rsqrt via the bitcast seed (in fp32 arithmetic;
        # the Newton step absorbs the rounding) + 1 Newton iteration --
        # avoids Ln on ScalarE so the whole kernel stays on one ACT table.
        i32 = mybir.dt.int32
        m = pool.tile([nrow, 512], f32, tag="lnm")
        t0 = pool.tile([nrow, 512], f32, tag="lnt")
        var = pool.tile([nrow, 512], f32, tag="lnv")
        y0i = pool.tile([nrow, 512], i32, tag="lny0")
        y1 = pool.tile([nrow, 512], f32, tag="lny1")
        nc.vector.tensor_scalar_mul(out=m, in0=s1_ps, scalar1=float(1.0 / n_tok))
        nc.vector.tensor_tensor(out=t0, in0=m, in1=s1_ps, op=Op.mult)
        nc.vector.tensor_tensor(out=var, in0=s2_ps, in1=t0, op=Op.subtract)
        nc.vector.tensor_scalar(out=y0i, in0=var.bitcast(i32), scalar1=-0.5,
                                scalar2=1597463007.0, op0=Op.mult, op1=Op.add)
        y0 = y0i.bitcast(f32)
        nc.vector.tensor_tensor(out=t0, in0=y0, in1=y0, op=Op.mult)
        nc.vector.tensor_tensor(out=t0, in0=t0, in1=var, op=Op.mult)
        nc.vector.tensor_scalar(out=t0, in0=t0, scalar1=-0.5, scalar2=1.5,
                                op0=Op.mult, op1=Op.add)
        nc.vector.tensor_tensor(out=y1, in0=y0, in1=t0, op=Op.mult)
        a = pool.tile([nrow, 512], bf16, tag="lna")
        c = pool.tile([nrow, 512], bf16, tag="lnc")
        nc.vector.tensor_scalar_mul(out=a, in0=y1, scalar1=alpha_ap)
        nc.vector.scalar_tensor_tensor(out=t0, in0=m, scalar=-1.0, in1=a,
                                       op0=Op.mult, op1=Op.mult)
        nc.vector.tensor_scalar_add(out=c, in0=t0, scalar1=beta_ap)
        return a, c

    a_rows, c_rows = ln_coeffs(p_ab, s1_ps, s2_ps,
                               consts_b[0:97, 0:1], consts_b[0:97, 1:2], 97, D_MODEL)

    # broadcast a/c rows to [P, S] via K=1 matmuls from partition 32t
    a_b = p_ab.tile([P, S], bf16)
    c_b = p_ab.tile([P, S], bf16)
    for rows, dst in ((a_rows, a_b), (c_rows, c_b)):
        for t in range(TC):
            b_ps = ps_mm.tile([P, 512], f32, tag="mm")
            mm(b_ps, ones_rows[32 * t:32 * t + 1, :], rows[32 * t:32 * t + 1, :],
               start=True, stop=True, tile_position=(32 * t, 0))
            nc.scalar.copy(out=dst[:, 512 * t:512 * (t + 1)], in_=b_ps)
    ps_st.release()

    # ---------------- LN1 apply -> fp8e4 ----------------
    ln1 = p_big.tile([P, KD, S], fp8e4, tag="ln1")

    def emit_apply(t):
        sl = slice(512 * t, 512 * (t + 1))
        eng = nc.gpsimd if t >= 2 else nc.vector
        for k in range(KD):
            sc = p_work.tile([P, 512], bf16, tag="lnsc_g" if t >= 2 else "lnsc")
            eng.tensor_tensor(out=sc, in0=xT_sb[:, t, k, :], in1=a_b[:, sl],
                              op=Op.mult)
            eng.tensor_tensor(out=ln1[:, k, sl], in0=sc, in1=c_b[:, sl],
                              op=Op.add)

    emit_apply(0)
    emit_apply(2)  # gpsimd, runs in parallel with everything
    emit_apply(3)  # gpsimd

    p_ab.release()
    p_xT.release()

    # ---------------- QKV projections (fp8e4 DoubleRow) ----------------
    qT = p_kv.tile([P, KD, Q], bf16)
    kT = p_kv.tile([P, KD, S], bf16)
    v_sb = p_kv.tile([P, TT, H, DKP], fp8e4)
    nc.gpsimd.memset(v_sb, 1.0)

    for j in range(KD):
        q_ps = ps_mm.tile([P, 512], f32, tag="mm")
        for k in range(2):
            mm(q_ps, wq_sb[:, 2 * k:2 * k + 2, j * P:(j + 1) * P],
               ln1[:, 2 * k:2 * k + 2, 0:Q],
               start=(k == 0), stop=(k == 1), perf_mode=DR)
        nc.vector.tensor_scalar(out=qT[:, j, :], in0=q_ps, scalar1=RSC,
                                scalar2=bq_sb[:, j:j + 1], op0=Op.mult, op1=Op.add)

    def emit_k_chunk(t):
        sl = slice(512 * t, 512 * (t + 1))
        for j in range(KD):
            k_ps = ps_mm.tile([P, 512], f32, tag="mm")
            for k in range(2):
                mm(k_ps, wk_sb[:, 2 * k:2 * k + 2, j * P:(j + 1) * P],
                   ln1[:, 2 * k:2 * k + 2, sl],
                   start=(k == 0), stop=(k == 1), perf_mode=DR)
            nc.scalar.activation(out=kT[:, j, sl], in_=k_ps, func=Act.Identity,
                                 bias=bk_sb[:, j:j + 1], scale=RSC)

    def emit_v_chunk(t):
        for tt in range(4 * t, 4 * t + 4):
            v_ps = ps_mm.tile([P, 512], f32, tag="mm")
            for k in range(2):
                mm(v_ps, ln1[:, 2 * k:2 * k + 2, tt * P:(tt + 1) * P],
                   wv_sb[:, 2 * k:2 * k + 2, :],
                   start=(k == 0), stop=(k == 1), perf_mode=DR)
            nc.vector.scalar_tensor_tensor(
                out=v_sb[:, tt, :, 0:DK],
                in0=v_ps.rearrange("p (h d) -> p h d", h=H),
                scalar=RSC,
                in1=bv_b.rearrange("p (h d) -> p h d", h=H),
                op0=Op.mult, op1=Op.add)

    # ---------------- QKV: all chunks before attention ----------------
    for t in range(TC):
        if t == 0:
            emit_apply(1)
        emit_k_chunk(t)
        emit_v_chunk(t)
    ps_mm.release()

    # W1 prefetch overlaps attention
    p_w1 = tc.alloc_tile_pool(name="p_w1", bufs=1, side="left")
    w1_sb = p_w1.tile([P, KD, D_FF], bf16)
    nc.sync.dma_start(out=w1_sb, in_=W1)
    b1_sb = p_w1.tile([P, FJ], f32)
    nc.sync.dma_start(out=b1_sb, in_=b1.rearrange("(j p) -> p j", p=P))

    # ---------------- attention: 2 passes x 2 head pairs ----------------
    # Two head pairs in flight keep the PE dense (scores of one pair run
    # while the other pair's exp drains) so HAM stays at K=8/8.
    p_attn = tc.alloc_tile_pool(name="p_attn", bufs=1, side="left")
    attn_sb = p_attn.tile([P, KD, Q], fp8e4)
    ps_sc = tc.alloc_tile_pool(name="ps_sc", bufs=3, space="PSUM")
    ps_ov = tc.alloc_tile_pool(name="ps_ov", bufs=1, space="PSUM")

    inv_sqrt_dk = float(1.0 / np.sqrt(np.float32(DK)))
    ln16b = p_small.tile([1, 1], f32)
    nc.vector.memset(ln16b, 90.8254037)  # ln(16) + 127.043*ln(2)

    def scores_and_exp(pj, tt, ex, par, engine):
        sc_ps = ps_sc.tile([P, 1024], f32, tag="sc")
        mm(sc_ps[:, 0:512],
           kT[0:DK, pj, tt * P:(tt + 1) * P], qT[0:DK, pj, :],
           start=True, stop=True, tile_position=(0, 0))
        mm(sc_ps[:, 512:1024],
           kT[DK:P, pj, tt * P:(tt + 1) * P], qT[DK:P, pj, :],
           start=True, stop=True, tile_position=(64, 0))
        if engine == 0:
            nc.scalar.activation(
                out=ex.bitcast(fp8e5)[:, par, :], in_=sc_ps,
                func=Act.Exp, bias=maskb8[:, tt:tt + 1], scale=inv_sqrt_dk)
        else:
            nc.vector.tensor_scalar(
                out=ex[:, par, :], in0=sc_ps,
                scalar1=float(A5 * inv_sqrt_dk), scalar2=maskb5[:, tt:tt + 1],
                op0=Op.mult, op1=Op.add)

    def emit_ov(pj, ov, ex, tp):
        ex5 = ex.bitcast(fp8e5)
        mm(ov[:, 0:512], v_sb[:, 2 * tp:2 * tp + 2, 2 * pj, 0:DK + 1],
           ex5[:, :, 0:512],
           start=(tp == 0), stop=(tp == TT // 2 - 1), perf_mode=DR)
        mm(ov[:, 512:1024], v_sb[:, 2 * tp:2 * tp + 2, 2 * pj + 1, 0:DK + 1],
           ex5[:, :, 512:1024],
           start=(tp == 0), stop=(tp == TT // 2 - 1), perf_mode=DR)

    def attn_norm(pj, ov_ps):
        # attn[:, pj] = 16 * ov[0:64] / ov[64]; copies free the PSUM tile
        # quickly (split across both engines) so the next pass can start.
        ovs = p_work2.tile([DK + 1, 1024], f32, tag="ovs")
        nc.scalar.copy(out=ovs[:, 0:512], in_=ov_ps[:, 0:512])
        nc.vector.tensor_copy(out=ovs[:, 512:1024], in_=ov_ps[:, 512:1024])
        rec = p_work2.tile([1, 1024], f32, tag="rec")
        # rec = exp(ln16 - ln(den)) = 16/den (+-3%) in ONE ACT op: the
        # int32-bitcast denominator approximates ln via its exponent bits.
        nc.scalar.activation(out=rec, in_=ovs[DK:DK + 1, :].bitcast(mybir.dt.int32),
                             func=Act.Exp, scale=-8.2629582e-8, bias=ln16b)
        rb = p_work2.tile([DK, 1024], f32, tag="rb")
        nc.gpsimd.partition_broadcast(out_ap=rb, in_ap=rec)
        for po in range(2):
            nc.vector.tensor_tensor(
                out=attn_sb[po * DK:(po + 1) * DK, pj, :],
                in0=ovs[0:DK, po * 512:(po + 1) * 512],
                in1=rb[:, po * 512:(po + 1) * 512], op=Op.mult)

    # ov matmuls are deferred one tp so the PE never waits on the just-
    # written exp tile -- the PE stream stays dense and HAM stays warm.
    for pj in range(H // 2):
        ov = ps_ov.tile([DK + 1, 1024], f32, tag="ov")
        pend = None
        for tp in range(TT // 2):
            ex = p_work.tile([P, 2, 1024], u8, tag="ex")
            for par in range(2):
                tt = 2 * tp + par
                eng = 0 if (par == 0 or tp == 4) else 1
                scores_and_exp(pj, tt, ex, par, engine=eng)
            if pend is not None:
                emit_ov(pj, ov, *pend)
            pend = (ex, tp)
        emit_ov(pj, ov, *pend)
        attn_norm(pj, ov)

    ps_ov.release()
    ps_sc.release()
    p_big.release()
    p_qkvw.release()
    ps_ffn = tc.alloc_tile_pool(name="ps_ffn", bufs=3, space="PSUM")
    ps_st2 = tc.alloc_tile_pool(name="ps_st2", bufs=2, space="PSUM")

    # W2 load overlaps Wo / LN2 / FFN-1
    p_w2 = tc.alloc_tile_pool(name="p_w2", bufs=1, side="right")
    w2_sb = p_w2.tile([P, FJ, D_MODEL], bf16)
    nc.sync.dma_start(out=w2_sb, in_=W2)
    b2_sb = p_w2.tile([P, KD], f32)
    nc.sync.dma_start(out=b2_sb, in_=b2.rearrange("(j p) -> p j", p=P))

    # ---------------- Wo + residual -> x2 (bf16) ----------------
    # bo enters via a K=1 rank-1 matmul (x256 to match the psum scale) so the
    # eviction is a single fused op; LN2 stats matmuls interleave per chunk.
    p_x2 = tc.alloc_tile_pool(name="p_x2", bufs=1, side="left")
    x2b = p_x2.tile([P, KD, Q], bf16)
    x2sq = p_x2.tile([P, KD, Q], bf16)
    bo256 = p_x2.tile([1, D_MODEL], bf16)
    nc.vector.tensor_scalar_mul(out=bo256, in0=bo_row, scalar1=float(WSC * WSC))
    s1q = ps_st2.tile([1, 512], f32, tag="st")
    s2q = ps_st2.tile([1, 512], f32, tag="st")
    for j in range(KD):
        o_ps = ps_ffn.tile([P, 512], f32, tag="mm")
        for k in range(2):
            mm(o_ps, wo_sb[:, 2 * k:2 * k + 2, j * P:(j + 1) * P],
               attn_sb[:, 2 * k:2 * k + 2, :],
               start=(k == 0), stop=False, perf_mode=DR)
        mm(o_ps, bo256[0:1, j * P:(j + 1) * P], ones_row512,
           start=False, stop=True, skip_group_check=True)
        nc.vector.scalar_tensor_tensor(out=x2b[:, j, :], in0=o_ps,
                                       scalar=float(RSC * RSC), in1=xq_sb[:, j, :],
                                       op0=Op.mult, op1=Op.add)
        nc.scalar.activation(out=x2sq[:, j, :], in_=x2b[:, j, :], func=Act.Square)
        mm(s1q, ones_col, x2b[:, j, :], start=(j == 0), stop=(j == KD - 1))
        mm(s2q, ones_col, x2sq[:, j, :], start=(j == 0), stop=(j == KD - 1))

    # ---------------- LN2 ----------------

    a2_row, c2_row = ln_coeffs(p_x2, s1q, s2q,
                               consts_sb[0:1, 2:3], consts_sb[0:1, 3:4], 1, D_MODEL)
    a2_b = p_x2.tile([P, Q], bf16)
    c2_b = p_x2.tile([P, Q], bf16)
    for row, dst in ((a2_row, a2_b), (c2_row, c2_b)):
        b_ps = ps_ffn.tile([P, 512], f32, tag="mm")
        mm(b_ps, ones_rows[0:1, :], row, start=True, stop=True)
        nc.scalar.copy(out=dst, in_=b_ps)

    ln2 = p_x2.tile([P, KD, Q], bf16)
    for k in range(KD):
        sc = p_work.tile([P, 512], bf16, tag="lnsc")
        nc.vector.tensor_tensor(out=sc, in0=x2b[:, k, :], in1=a2_b, op=Op.mult)
        nc.vector.tensor_tensor(out=ln2[:, k, :], in0=sc, in1=c2_b, op=Op.add)

    # ---------------- FFN ----------------
    p_h = tc.alloc_tile_pool(name="p_h", bufs=1, side="left")
    hT = p_h.tile([P, FJ, Q], bf16)
    for j in range(FJ):
        h_ps = ps_ffn.tile([P, 512], f32, tag="mm")
        for k in range(KD):
            mm(h_ps, w1_sb[:, k, j * P:(j + 1) * P], ln2[:, k, :],
               start=(k == 0), stop=(k == KD - 1))
        nc.scalar.activation(out=hT[:, j, :], in_=h_ps, func=Act.Relu,
                             bias=b1_sb[:, j:j + 1], scale=1.0)

    for j in range(KD):
        f_ps = ps_ffn.tile([P, 512], f32, tag="mm")
        for k in range(FJ):
            mm(f_ps, w2_sb[:, k, j * P:(j + 1) * P], hT[:, k, :],
               start=(k == 0), stop=(k == FJ - 1))
        o_sb = p_work.tile([P, 512], f32, tag="osb")
        nc.vector.scalar_tensor_tensor(out=o_sb, in0=f_ps,
                                       scalar=b2_sb[:, j:j + 1], in1=x2b[:, j, :],
                                       op0=Op.add, op1=Op.add)
        outr = outT.rearrange("(j p) q -> p j q", p=P)
        nc.sync.dma_start(out=outr[:, j, 0:256], in_=o_sb[:, 0:256])
        nc.sync.dma_start(out=outr[:, j, 256:512], in_=o_sb[:, 256:512])

    for pool in (p_h, p_x2, p_attn, p_w1, p_kv, p_work2, p_work, p_small,
                 ps_st2, ps_ffn, p_w2, p_xq, p_wo):
        pool.release()


def _build():
    global _BUILT
    if _BUILT is not None:
        return _BUILT
    import concourse.bass as bass
    import concourse.tile as tile
    from concourse import bacc, mybir

    f32 = mybir.dt.float32
    bf16 = mybir.dt.bfloat16
    i32 = mybir.dt.int32
    fp8e4 = mybir.dt.float8e4
    nc = bacc.Bacc(
        "TRN2",
        target_bir_lowering=False,
        debug=False,
        enable_asserts=False,
        num_devices=N_CORES,
    )
    aps = [
        nc.dram_tensor("xT", [P, TC, KD, 512], bf16, kind="ExternalInput").ap(),
        nc.dram_tensor("xq", [P, KD, Q], f32, kind="ExternalInput").ap(),
        nc.dram_tensor("mask", [S], i32, kind="ExternalInput").ap(),
        nc.dram_tensor("Wq", [P, KD, D_MODEL], fp8e4, kind="ExternalInput").ap(),
        nc.dram_tensor("Wk", [P, KD, D_MODEL], fp8e4, kind="ExternalInput").ap(),
        nc.dram_tensor("Wv", [P, KD, D_MODEL], fp8e4, kind="ExternalInput").ap(),
        nc.dram_tensor("Wo", [P, KD, D_MODEL], fp8e4, kind="ExternalInput").ap(),
        nc.dram_tensor("W1", [P, KD, D_FF], bf16, kind="ExternalInput").ap(),
        nc.dram_tensor("W2", [P, FJ, D_MODEL], bf16, kind="ExternalInput").ap(),
        nc.dram_tensor("bq", [D_MODEL], f32, kind="ExternalInput").ap(),
        nc.dram_tensor("bk", [D_MODEL], f32, kind="ExternalInput").ap(),
        nc.dram_tensor("bv", [D_MODEL], f32, kind="ExternalInput").ap(),
        nc.dram_tensor("bo", [D_MODEL], f32, kind="ExternalInput").ap(),
        nc.dram_tensor("b1", [D_FF], f32, kind="ExternalInput").ap(),
        nc.dram_tensor("b2", [D_MODEL], f32, kind="ExternalInput").ap(),
        nc.dram_tensor("consts", [4], f32, kind="ExternalInput").ap(),
        nc.dram_tensor("outT", [D_MODEL, Q], f32, kind="ExternalOutput").ap(),
    ]
    with tile.TileContext(nc) as tc:
        _emit(nc, tc, aps)
    nc.compile()
    _BUILT = nc
    return nc


def make_in_maps(inputs):
    import ml_dtypes

    bf16 = ml_dtypes.bfloat16
    e4 = ml_dtypes.float8_e4m3
    x = np.asarray(inputs["x"], np.float32)
    src_mask = np.asarray(inputs["src_mask"], np.int32)

    def w8(name):
        # [D_MODEL, out] -> [P, KD, out] with feature f = k*P + p
        w = np.asarray(inputs[name], np.float32) * WSC
        w = w.reshape(KD, P, -1).transpose(1, 0, 2)
        return np.ascontiguousarray(w.astype(e4)).view(np.uint8)

    shared = {
        "Wq": w8("Wq"), "Wk": w8("Wk"), "Wv": w8("Wv"), "Wo": w8("Wo"),
        "W1": np.ascontiguousarray(np.asarray(inputs["W1"], np.float32)
                                   .reshape(KD, P, D_FF).transpose(1, 0, 2).astype(bf16)),
        "W2": np.ascontiguousarray(np.asarray(inputs["W2"], np.float32)
                                   .reshape(FJ, P, D_MODEL).transpose(1, 0, 2).astype(bf16)),
        "bq": np.ascontiguousarray(np.asarray(inputs["bq"], np.float32)),
        "bk": np.ascontiguousarray(np.asarray(inputs["bk"], np.float32)),
        "bv": np.ascontiguousarray(np.asarray(inputs["bv"], np.float32)),
        "bo": np.ascontiguousarray(np.asarray(inputs["bo"], np.float32)),
        "b1": np.ascontiguousarray(np.asarray(inputs["b1"], np.float32)),
        "b2": np.ascontiguousarray(np.asarray(inputs["b2"], np.float32)),
        "consts": np.ascontiguousarray(
            np.array(
                [
                    np.asarray(inputs["alpha1"]).reshape(-1)[0] * np.sqrt(511.0),
                    np.asarray(inputs["beta1"]).reshape(-1)[0],
                    np.asarray(inputs["alpha2"]).reshape(-1)[0] * np.sqrt(511.0),
                    np.asarray(inputs["beta2"]).reshape(-1)[0],
                ],
                np.float32,
            )
        ),
    }
    in_maps = []
    for c in range(N_CORES):
        b = c // CORES_PER_BATCH
        qs = (c % CORES_PER_BATCH) * Q
        x_rot = np.concatenate([x[b, qs:, :], x[b, :qs, :]], axis=0)
        m_b = src_mask[b, 0, 0, :]
        m_rot = np.concatenate([m_b[qs:], m_b[:qs]], axis=0)
        in_map = dict(shared)
        # xT[p, t, k, tt] = x_rot[512t+tt, 128k+p]
        xr = x_rot.reshape(TC, 512, KD, P)
        in_map["xT"] = np.ascontiguousarray(
            xr.transpose(3, 0, 2, 1).astype(bf16))
        # xq[p, k, t] = x_rot[t, 128k+p]
        in_map["xq"] = np.ascontiguousarray(
            x_rot[0:Q, :].reshape(Q, KD, P).transpose(2, 1, 0))
        in_map["mask"] = np.ascontiguousarray(m_rot)
        in_maps.append(in_map)
    return in_maps


def assemble_output(results):
    out = np.empty((B, S, D_MODEL), np.float32)
    for c in range(N_CORES):
        b = c // CORES_PER_BATCH
        qs = (c % CORES_PER_BATCH) * Q
        out[b, qs:qs + Q, :] = results[c]["outT"].T
    return out


def kernel(**inputs):
    from concourse.bass_utils import run_bass_kernel_spmd

    nc = _build()
    in_maps = make_in_maps(inputs)
    res = run_bass_kernel_spmd(nc, in_maps, core_ids=list(range(N_CORES)))
    return assemble_output(res.results)


# revision 54
# speedup vs baseline: 1.0212x; 1.0212x over previous
# Trainium2 Bass kernel for nn_EncoderBlock (dense transformer encoder block).
#
# Sharding: 8 cores, zero collectives. Core c owns batch b = c // 4 and query
# slice qs = (c % 4) * 512. Each core redundantly computes LN1/K/V for its
# whole batch (2048 tokens) and runs attention + FFN for its own 512 queries.
# The host rolls the token order per core so that the core's queries are
# tokens 0..511 of its view -- every core runs the identical SPMD program.
#
# v2 highlights over the bf16 baseline:
#  - QKV/Wo projections in fp8e4 DoubleRow (weights x16 host-prescaled,
#    evictions rescale by 1/16): half the matmul instructions.
#  - attn@v in mixed fp8 DoubleRow: v in fp8e4, softmax weights as fp8e5
#    bits; kpos chunks are consumed in pairs (contraction 256 per pass).
#  - softmax exp split across ScalarE (ACT Exp) and VectorE (Schraudolph
#    bitcast exp: bits = x*A5 + B5 -> uint8, saturation implements the mask).
#  - single ACT table set (ln/exp/identity/relu/square) -> one table load.
#  - LN stats for all four 512-token chunks are col-tiled into one [97,512]
#    PSUM tile (partitions 0/32/64/96) so the coeff chain runs once.
#  - chunk-pipelined front; attention runs in four passes (one head pair
#    each) to fit PSUM: 2 (qkv) + 4 (scores) + 2 (ov) banks.

import numpy as np

D_MODEL = 512
H = 8
DK = 64
DKP = 72          # padded per-head v row (65 used: 64 + ones column)
D_FF = 2048
B = 2
S = 2048
EPS = 1e-6

N_CORES = 8
CORES_PER_BATCH = 4
Q = 512          # queries per core
P = 128          # partitions
KD = D_MODEL // P      # 4 feature chunks
FJ = D_FF // P         # 16 ff chunks
TT = S // P            # 16 kpos chunks
TC = S // 512          # 4 token column chunks

A5 = 5.770780          # 4 / ln2      (e5m2 schraudolph scale)
B5 = 59.768            # 15*4 - 0.232 (e5m2 schraudolph offset)
WSC = 16.0             # host prescale on fp8 weights
RSC = float(1.0 / WSC)

_BUILT = None


def _emit(nc, tc, aps):
    import concourse.bass as bass
    from concourse import mybir

    f32 = mybir.dt.float32
    bf16 = mybir.dt.bfloat16
    u8 = mybir.dt.uint8
    fp8e4 = mybir.dt.float8e4
    fp8e5 = mybir.dt.float8e5
    Act = mybir.ActivationFunctionType
    Op = mybir.AluOpType
    DR = mybir.MatmulPerfMode.DoubleRow

    (xT, xq, mask, Wq, Wk, Wv, Wo, W1, W2,
     bq, bk, bv, bo, b1, b2, consts, outT) = aps

    mm = nc.tensor.matmul

    # ---------------- pools ----------------
    # LEFT stack : small, work | kv | w1, x2, h
    # RIGHT stack: wo | xq | qkvw | big(ln1) | xT | ab
    p_small = tc.alloc_tile_pool(name="p_small", bufs=1, side="left")
    p_work = tc.alloc_tile_pool(name="p_work", bufs=3, side="left")
    p_work2 = tc.alloc_tile_pool(name="p_work2", bufs=2, side="left")
    p_kv = tc.alloc_tile_pool(name="p_kv", bufs=1, side="left")
    p_wo = tc.alloc_tile_pool(name="p_wo", bufs=1, side="right")
    p_xq = tc.alloc_tile_pool(name="p_xq", bufs=1, side="right")
    p_qkvw = tc.alloc_tile_pool(name="p_qkvw", bufs=1, side="right")
    p_big = tc.alloc_tile_pool(name="p_big", bufs=1, side="right")
    p_xT = tc.alloc_tile_pool(name="p_xT", bufs=1, side="right")
    p_ab = tc.alloc_tile_pool(name="p_ab", bufs=1, side="right")

    ps_mm = tc.alloc_tile_pool(name="ps_mm", bufs=4, space="PSUM")
    ps_st = tc.alloc_tile_pool(name="ps_st", bufs=2, space="PSUM")

    # ---------------- constant / input loads ----------------
    ones_col = p_small.tile([P, 1], bf16)
    nc.vector.memset(ones_col, 1.0)
    ones_rows = p_small.tile([P, P], bf16)
    nc.vector.memset(ones_rows, 1.0)
    ones_row512 = p_small.tile([1, 512], bf16)
    nc.vector.memset(ones_row512, 1.0)
    warm_sb = p_small.tile([P, 512], bf16)
    nc.vector.memset(warm_sb, 0.0)

    # x (transposed, rotated, fp8), chunk-major so each chunk DMA is one
    # contiguous 2KB descriptor per partition; chunk 0 lands first
    xT_sb = p_xT.tile([P, TC, KD, 512], bf16)
    nc.sync.dma_start(out=xT_sb[:, 0], in_=xT[:, 0])
    nc.sync.dma_start(out=xT_sb[:, 1:TC], in_=xT[:, 1:TC])

    wq_sb = p_qkvw.tile([P, KD, D_MODEL], fp8e4)
    wk_sb = p_qkvw.tile([P, KD, D_MODEL], fp8e4)
    wv_sb = p_qkvw.tile([P, KD, D_MODEL], fp8e4)
    wo_sb = p_wo.tile([P, KD, D_MODEL], fp8e4)
    for w_sb, w in ((wq_sb, Wq), (wk_sb, Wk), (wv_sb, Wv), (wo_sb, Wo)):
        nc.sync.dma_start(out=w_sb, in_=w)
    bq_sb = p_qkvw.tile([P, KD], f32)
    bk_sb = p_qkvw.tile([P, KD], f32)
    bo_row = p_wo.tile([1, D_MODEL], f32)
    nc.sync.dma_start(out=bq_sb, in_=bq.rearrange("(j p) -> p j", p=P))
    nc.sync.dma_start(out=bk_sb, in_=bk.rearrange("(j p) -> p j", p=P))
    nc.sync.dma_start(out=bo_row, in_=bo.rearrange("(o d) -> o d", o=1))
    bv_b = p_qkvw.tile([P, D_MODEL], f32)
    nc.sync.dma_start(
        out=bv_b, in_=bv.rearrange("(o d) -> o d", o=1).to_broadcast([P, D_MODEL])
    )
    xq_sb = p_xq.tile([P, KD, Q], f32)
    nc.sync.dma_start(out=xq_sb, in_=xq)

    consts_sb = p_small.tile([1, 4], f32)
    nc.sync.dma_start(out=consts_sb, in_=consts.rearrange("(o c) -> o c", o=1))
    consts_b = p_small.tile([P, 4], f32)
    nc.gpsimd.partition_broadcast(out_ap=consts_b, in_ap=consts_sb)

    mask_i = p_small.tile([P, TT], mybir.dt.int32)
    nc.sync.dma_start(out=mask_i, in_=mask.rearrange("(t p) -> p t", p=P))
    mask_f = p_small.tile([P, TT], f32)
    nc.vector.tensor_copy(out=mask_f, in_=mask_i)
    # ACT-exp bias: mask 1 -> 0 ; mask 0 -> -1e9
    maskb8 = p_small.tile([P, TT], f32)
    nc.vector.tensor_scalar(out=maskb8, in0=mask_f, scalar1=1e9, scalar2=-1e9,
                            op0=Op.mult, op1=Op.add)
    # schraudolph bias: mask 1 -> B5 ; mask 0 -> B5 - 1e9 (saturates to 0)
    maskb5 = p_small.tile([P, TT], f32)
    nc.vector.tensor_scalar(out=maskb5, in0=mask_f, scalar1=1e9,
                            scalar2=float(B5 - 1e9), op0=Op.mult, op1=Op.add)


    # ---------------- LN1 stats (col-tiled into [97, 512]) ----------------
    xsq = p_big.tile([P, TC, KD, 512], bf16, tag="xsq")
    for t in range(TC):
        for k in range(KD):
            if t < 2:
                nc.vector.tensor_tensor(out=xsq[:, t, k, :], in0=xT_sb[:, t, k, :],
                                        in1=xT_sb[:, t, k, :], op=Op.mult)
            else:
                nc.scalar.activation(out=xsq[:, t, k, :], in_=xT_sb[:, t, k, :],
                                     func=Act.Square)

    # PE warm-up: dummy matmuls run during the input DMA wait, so the HAM
    # clock gate is at K=8/8 when the stats matmuls arrive.
    warm_ps = ps_mm.tile([P, 512], f32, tag="mm")
    for w in range(18):
        mm(warm_ps, ones_rows, warm_sb, start=(w == 0), stop=(w == 17),
           skip_group_check=True)

    s1_ps = ps_st.tile([97, 512], f32, tag="st")
    s2_ps = ps_st.tile([97, 512], f32, tag="st")
    for t in range(TC):
        for k in range(KD):
            mm(s1_ps[32 * t:32 * t + 1, :], ones_col, xT_sb[:, t, k, :],
               start=(k == 0), stop=(k == KD - 1), tile_position=(0, 32 * t))
    for t in range(TC):
        for k in range(KD):
            mm(s2_ps[32 * t:32 * t + 1, :], ones_col, xsq[:, t, k, :],
               start=(k == 0), stop=(k == KD - 1), tile_position=(0, 32 * t))

    def ln_coeffs(pool, s1_ps, s2_ps, alpha_ap, beta_ap, nrow, n_tok):
        # m = s1/n; var = (s2 - m*s1)/(n-1); a = alpha*rsqrt(var);
        # c = beta - m*a.  rsqrt via the bitcast seed (in fp32 arithmetic;
        # the Newton step absorbs the rounding) + 1 Newton iteration --
        # avoids Ln on ScalarE so the whole kernel stays on one ACT table.
        i32 = mybir.dt.int32
        m = pool.tile([nrow, 512], f32, tag="lnm")
        t0 = pool.tile([nrow, 512], f32, tag="lnt")
        var = pool.tile([nrow, 512], f32, tag="lnv")
        y0i = pool.tile([nrow, 512], i32, tag="lny0")
        y1 = pool.tile([nrow, 512], f32, tag="lny1")
        nc.vector.tensor_scalar_mul(out=m, in0=s1_ps, scalar1=float(1.0 / n_tok))
        nc.vector.tensor_tensor(out=t0, in0=m, in1=s1_ps, op=Op.mult)
        nc.vector.tensor_tensor(out=var, in0=s2_ps, in1=t0, op=Op.subtract)
        nc.vector.tensor_scalar(out=y0i, in0=var.bitcast(i32), scalar1=-0.5,
                                scalar2=1597463007.0, op0=Op.mult, op1=Op.add)
        y0 = y0i.bitcast(f32)
        nc.vector.tensor_tensor(out=t0, in0=y0, in1=y0, op=Op.mult)
        nc.vector.tensor_tensor(out=t0, in0=t0, in1=var, op=Op.mult)
        nc.vector.tensor_scalar(out=t0, in0=t0, scalar1=-0.5, scalar2=1.5,
                                op0=Op.mult, op1=Op.add)
        nc.vector.tensor_tensor(out=y1, in0=y0, in1=t0, op=Op.mult)
        a = pool.tile([nrow, 512], bf16, tag="lna")
        c = pool.tile([nrow, 512], bf16, tag="lnc")
        nc.vector.tensor_scalar_mul(out=a, in0=y1, scalar1=alpha_ap)
        nc.vector.scalar_tensor_tensor(out=t0, in0=m, scalar=-1.0, in1=a,
                                       op0=Op.mult, op1=Op.mult)
        nc.vector.tensor_scalar_add(out=c, in0=t0, scalar1=beta_ap)
        return a, c

    a_rows, c_rows = ln_coeffs(p_ab, s1_ps, s2_ps,
                               consts_b[0:97, 0:1], consts_b[0:97, 1:2], 97, D_MODEL)

    # broadcast a/c rows to [P, S] via K=1 matmuls from partition 32t
    a_b = p_ab.tile([P, S], bf16)
    c_b = p_ab.tile([P, S], bf16)
    for t in range(TC):
        for rows, dst in ((a_rows, a_b), (c_rows, c_b)):
            b_ps = ps_mm.tile([P, 512], f32, tag="mm")
            mm(b_ps, ones_rows[32 * t:32 * t + 1, :], rows[32 * t:32 * t + 1, :],
               start=True, stop=True, tile_position=(32 * t, 0))
            nc.scalar.copy(out=dst[:, 512 * t:512 * (t + 1)], in_=b_ps)
    ps_st.release()

    # ---------------- LN1 apply -> fp8e4 ----------------
    ln1 = p_big.tile([P, KD, S], fp8e4, tag="ln1")

    def emit_apply(t):
        sl = slice(512 * t, 512 * (t + 1))
        eng = nc.gpsimd if t >= 2 else nc.vector
        for k in range(KD):
            sc = p_work.tile([P, 512], bf16, tag="lnsc_g" if t >= 2 else "lnsc")
            eng.tensor_tensor(out=sc, in0=xT_sb[:, t, k, :], in1=a_b[:, sl],
                              op=Op.mult)
            eng.tensor_tensor(out=ln1[:, k, sl], in0=sc, in1=c_b[:, sl],
                              op=Op.add)

    emit_apply(0)
    emit_apply(2)  # gpsimd, runs in parallel with everything
    emit_apply(3)  # gpsimd

    p_ab.release()
    p_xT.release()

    # ---------------- QKV projections (fp8e4 DoubleRow) ----------------
    qT = p_kv.tile([P, KD, Q], bf16)
    kT = p_kv.tile([P, KD, S], bf16)
    v_sb = p_kv.tile([P, TT, H, DKP], fp8e4)
    nc.gpsimd.memset(v_sb, 1.0)

    for j in range(KD):
        q_ps = ps_mm.tile([P, 512], f32, tag="mm")
        for k in range(2):
            mm(q_ps, wq_sb[:, 2 * k:2 * k + 2, j * P:(j + 1) * P],
               ln1[:, 2 * k:2 * k + 2, 0:Q],
               start=(k == 0), stop=(k == 1), perf_mode=DR)
        nc.vector.tensor_scalar(out=qT[:, j, :], in0=q_ps, scalar1=RSC,
                                scalar2=bq_sb[:, j:j + 1], op0=Op.mult, op1=Op.add)

    def emit_k_chunk(t):
        sl = slice(512 * t, 512 * (t + 1))
        for j in range(KD):
            k_ps = ps_mm.tile([P, 512], f32, tag="mm")
            for k in range(2):
                mm(k_ps, wk_sb[:, 2 * k:2 * k + 2, j * P:(j + 1) * P],
                   ln1[:, 2 * k:2 * k + 2, sl],
                   start=(k == 0), stop=(k == 1), perf_mode=DR)
            nc.scalar.activation(out=kT[:, j, sl], in_=k_ps, func=Act.Identity,
                                 bias=bk_sb[:, j:j + 1], scale=RSC)

    def emit_v_chunk(t):
        for tt in range(4 * t, 4 * t + 4):
            v_ps = ps_mm.tile([P, 512], f32, tag="mm")
            for k in range(2):
                mm(v_ps, ln1[:, 2 * k:2 * k + 2, tt * P:(tt + 1) * P],
                   wv_sb[:, 2 * k:2 * k + 2, :],
                   start=(k == 0), stop=(k == 1), perf_mode=DR)
            nc.vector.scalar_tensor_tensor(
                out=v_sb[:, tt, :, 0:DK],
                in0=v_ps.rearrange("p (h d) -> p h d", h=H),
                scalar=RSC,
                in1=bv_b.rearrange("p (h d) -> p h d", h=H),
                op0=Op.mult, op1=Op.add)

    # ---------------- QKV: all chunks before attention ----------------
    for t in range(TC):
        if t == 0:
            emit_apply(1)
        emit_k_chunk(t)
        emit_v_chunk(t)
    ps_mm.release()

    # W1 prefetch overlaps attention
    p_w1 = tc.alloc_tile_pool(name="p_w1", bufs=1, side="left")
    w1_sb = p_w1.tile([P, KD, D_FF], bf16)
    nc.sync.dma_start(out=w1_sb, in_=W1)
    b1_sb = p_w1.tile([P, FJ], f32)
    nc.sync.dma_start(out=b1_sb, in_=b1.rearrange("(j p) -> p j", p=P))

    # ---------------- attention: 2 passes x 2 head pairs ----------------
    # Two head pairs in flight keep the PE dense (scores of one pair run
    # while the other pair's exp drains) so HAM stays at K=8/8.
    p_attn = tc.alloc_tile_pool(name="p_attn", bufs=1, side="left")
    attn_sb = p_attn.tile([P, KD, Q], fp8e4)
    ps_sc = tc.alloc_tile_pool(name="ps_sc", bufs=3, space="PSUM")
    ps_ov = tc.alloc_tile_pool(name="ps_ov", bufs=1, space="PSUM")

    inv_sqrt_dk = float(1.0 / np.sqrt(np.float32(DK)))
    ln16b = p_small.tile([1, 1], f32)
    nc.vector.memset(ln16b, 90.8254037)  # ln(16) + 127.043*ln(2)

    def scores_and_exp(pj, tt, ex, par, engine):
        sc_ps = ps_sc.tile([P, 1024], f32, tag="sc")
        mm(sc_ps[:, 0:512],
           kT[0:DK, pj, tt * P:(tt + 1) * P], qT[0:DK, pj, :],
           start=True, stop=True, tile_position=(0, 0))
        mm(sc_ps[:, 512:1024],
           kT[DK:P, pj, tt * P:(tt + 1) * P], qT[DK:P, pj, :],
           start=True, stop=True, tile_position=(64, 0))
        if engine == 0:
            nc.scalar.activation(
                out=ex.bitcast(fp8e5)[:, par, :], in_=sc_ps,
                func=Act.Exp, bias=maskb8[:, tt:tt + 1], scale=inv_sqrt_dk)
        else:
            nc.vector.tensor_scalar(
                out=ex[:, par, :], in0=sc_ps,
                scalar1=float(A5 * inv_sqrt_dk), scalar2=maskb5[:, tt:tt + 1],
                op0=Op.mult, op1=Op.add)

    def emit_ov(pj, ov, ex, tp):
        ex5 = ex.bitcast(fp8e5)
        mm(ov[:, 0:512], v_sb[:, 2 * tp:2 * tp + 2, 2 * pj, 0:DK + 1],
           ex5[:, :, 0:512],
           start=(tp == 0), stop=(tp == TT // 2 - 1), perf_mode=DR)
        mm(ov[:, 512:1024], v_sb[:, 2 * tp:2 * tp + 2, 2 * pj + 1, 0:DK + 1],
           ex5[:, :, 512:1024],
           start=(tp == 0), stop=(tp == TT // 2 - 1), perf_mode=DR)

    def attn_norm(pj, ov_ps):
        # attn[:, pj] = 16 * ov[0:64] / ov[64]; copies free the PSUM tile
        # quickly (split across both engines) so the next pass can start.
        ovs = p_work2.tile([DK + 1, 1024], f32, tag="ovs")
        nc.scalar.copy(out=ovs[:, 0:512], in_=ov_ps[:, 0:512])
        nc.vector.tensor_copy(out=ovs[:, 512:1024], in_=ov_ps[:, 512:1024])
        rec = p_work2.tile([1, 1024], f32, tag="rec")
        # rec = exp(ln16 - ln(den)) = 16/den (+-3%) in ONE ACT op: the
        # int32-bitcast denominator approximates ln via its exponent bits.
        nc.scalar.activation(out=rec, in_=ovs[DK:DK + 1, :].bitcast(mybir.dt.int32),
                             func=Act.Exp, scale=-8.2629582e-8, bias=ln16b)
        rb = p_work2.tile([DK, 1024], f32, tag="rb")
        nc.gpsimd.partition_broadcast(out_ap=rb, in_ap=rec)
        for po in range(2):
            nc.vector.tensor_tensor(
                out=attn_sb[po * DK:(po + 1) * DK, pj, :],
                in0=ovs[0:DK, po * 512:(po + 1) * 512],
                in1=rb[:, po * 512:(po + 1) * 512], op=Op.mult)

    # ov matmuls are deferred one tp so the PE never waits on the just-
    # written exp tile -- the PE stream stays dense and HAM stays warm.
    for pj in range(H // 2):
        ov = ps_ov.tile([DK + 1, 1024], f32, tag="ov")
        pend = None
        for tp in range(TT // 2):
            ex = p_work.tile([P, 2, 1024], u8, tag="ex")
            for par in range(2):
                tt = 2 * tp + par
                eng = 0 if (par == 0 or tp == 4) else 1
                scores_and_exp(pj, tt, ex, par, engine=eng)
            if pend is not None:
                emit_ov(pj, ov, *pend)
            pend = (ex, tp)
        emit_ov(pj, ov, *pend)
        attn_norm(pj, ov)

    ps_ov.release()
    ps_sc.release()
    p_big.release()
    p_qkvw.release()
    ps_ffn = tc.alloc_tile_pool(name="ps_ffn", bufs=3, space="PSUM")
    ps_st2 = tc.alloc_tile_pool(name="ps_st2", bufs=2, space="PSUM")

    # W2 load overlaps Wo / LN2 / FFN-1
    p_w2 = tc.alloc_tile_pool(name="p_w2", bufs=1, side="right")
    w2_sb = p_w2.tile([P, FJ, D_MODEL], bf16)
    nc.sync.dma_start(out=w2_sb, in_=W2)
    b2_sb = p_w2.tile([P, KD], f32)
    nc.sync.dma_start(out=b2_sb, in_=b2.rearrange("(j p) -> p j", p=P))

    # ---------------- Wo + residual -> x2 (bf16) ----------------
    # bo enters via a K=1 rank-1 matmul (x256 to match the psum scale) so the
    # eviction is a single fused op; LN2 stats matmuls interleave per chunk.
    p_x2 = tc.alloc_tile_pool(name="p_x2", bufs=1, side="left")
    x2b = p_x2.tile([P, KD, Q], bf16)
    x2sq = p_x2.tile([P, KD, Q], bf16)
    bo256 = p_x2.tile([1, D_MODEL], bf16)
    nc.vector.tensor_scalar_mul(out=bo256, in0=bo_row, scalar1=float(WSC * WSC))
    s1q = ps_st2.tile([1, 512], f32, tag="st")
    s2q = ps_st2.tile([1, 512], f32, tag="st")
    for j in range(KD):
        o_ps = ps_ffn.tile([P, 512], f32, tag="mm")
        for k in range(2):
            mm(o_ps, wo_sb[:, 2 * k:2 * k + 2, j * P:(j + 1) * P],
               attn_sb[:, 2 * k:2 * k + 2, :],
               start=(k == 0), stop=False, perf_mode=DR)
        mm(o_ps, bo256[0:1, j * P:(j + 1) * P], ones_row512,
           start=False, stop=True, skip_group_check=True)
        nc.vector.scalar_tensor_tensor(out=x2b[:, j, :], in0=o_ps,
                                       scalar=float(RSC * RSC), in1=xq_sb[:, j, :],
                                       op0=Op.mult, op1=Op.add)
        nc.scalar.activation(out=x2sq[:, j, :], in_=x2b[:, j, :], func=Act.Square)
        mm(s1q, ones_col, x2b[:, j, :], start=(j == 0), stop=(j == KD - 1))
        mm(s2q, ones_col, x2sq[:, j, :], start=(j == 0), stop=(j == KD - 1))

    # ---------------- LN2 ----------------

    a2_row, c2_row = ln_coeffs(p_x2, s1q, s2q,
                               consts_sb[0:1, 2:3], consts_sb[0:1, 3:4], 1, D_MODEL)
    a2_b = p_x2.tile([P, Q], bf16)
    c2_b = p_x2.tile([P, Q], bf16)
    for row, dst in ((a2_row, a2_b), (c2_row, c2_b)):
        b_ps = ps_ffn.tile([P, 512], f32, tag="mm")
        mm(b_ps, ones_rows[0:1, :], row, start=True, stop=True)
        nc.scalar.copy(out=dst, in_=b_ps)

    ln2 = p_x2.tile([P, KD, Q], bf16)
    for k in range(KD):
        sc = p_work.tile([P, 512], bf16, tag="lnsc")
        nc.vector.tensor_tensor(out=sc, in0=x2b[:, k, :], in1=a2_b, op=Op.mult)
        nc.vector.tensor_tensor(out=ln2[:, k, :], in0=sc, in1=c2_b, op=Op.add)

    # ---------------- FFN ----------------
    p_h = tc.alloc_tile_pool(name="p_h", bufs=1, side="left")
    hT = p_h.tile([P, FJ, Q], bf16)
    for j in range(FJ):
        h_ps = ps_ffn.tile([P, 512], f32, tag="mm")
        for k in range(KD):
            mm(h_ps, w1_sb[:, k, j * P:(j + 1) * P], ln2[:, k, :],
               start=(k == 0), stop=(k == KD - 1))
        nc.scalar.activation(out=hT[:, j, :], in_=h_ps, func=Act.Relu,
                             bias=b1_sb[:, j:j + 1], scale=1.0)

    for j in range(KD):
        f_ps = ps_ffn.tile([P, 512], f32, tag="mm")
        for k in range(FJ):
            mm(f_ps, w2_sb[:, k, j * P:(j + 1) * P], hT[:, k, :],
               start=(k == 0), stop=(k == FJ - 1))
        o_sb = p_work.tile([P, 512], f32, tag="osb")
        nc.vector.scalar_tensor_tensor(out=o_sb, in0=f_ps,
                                       scalar=b2_sb[:, j:j + 1], in1=x2b[:, j, :],
                                       op0=Op.add, op1=Op.add)
        outr = outT.rearrange("(j p) q -> p j q", p=P)
        nc.sync.dma_start(out=outr[:, j, 0:256], in_=o_sb[:, 0:256])
        nc.sync.dma_start(out=outr[:, j, 256:512], in_=o_sb[:, 256:512])

    for pool in (p_h, p_x2, p_attn, p_w1, p_kv, p_work2, p_work, p_small,
                 ps_st2, ps_ffn, p_w2, p_xq, p_wo):
        pool.release()


def _build():
    global _BUILT
    if _BUILT is not None:
        return _BUILT
    import concourse.bass as bass
    import concourse.tile as tile
    from concourse import bacc, mybir

    f32 = mybir.dt.float32
    bf16 = mybir.dt.bfloat16
    i32 = mybir.dt.int32
    fp8e4 = mybir.dt.float8e4
    nc = bacc.Bacc(
        "TRN2",
        target_bir_lowering=False,
        debug=False,
        enable_asserts=False,
        num_devices=N_CORES,
    )
    aps = [
        nc.dram_tensor("xT", [P, TC, KD, 512], bf16, kind="ExternalInput").ap(),
        nc.dram_tensor("xq", [P, KD, Q], f32, kind="ExternalInput").ap(),
        nc.dram_tensor("mask", [S], i32, kind="ExternalInput").ap(),
        nc.dram_tensor("Wq", [P, KD, D_MODEL], fp8e4, kind="ExternalInput").ap(),
        nc.dram_tensor("Wk", [P, KD, D_MODEL], fp8e4, kind="ExternalInput").ap(),
        nc.dram_tensor("Wv", [P, KD, D_MODEL], fp8e4, kind="ExternalInput").ap(),
        nc.dram_tensor("Wo", [P, KD, D_MODEL], fp8e4, kind="ExternalInput").ap(),
        nc.dram_tensor("W1", [P, KD, D_FF], bf16, kind="ExternalInput").ap(),
        nc.dram_tensor("W2", [P, FJ, D_MODEL], bf16, kind="ExternalInput").ap(),
        nc.dram_tensor("bq", [D_MODEL], f32, kind="ExternalInput").ap(),
        nc.dram_tensor("bk", [D_MODEL], f32, kind="ExternalInput").ap(),
        nc.dram_tensor("bv", [D_MODEL], f32, kind="ExternalInput").ap(),
        nc.dram_tensor("bo", [D_MODEL], f32, kind="ExternalInput").ap(),
        nc.dram_tensor("b1", [D_FF], f32, kind="ExternalInput").ap(),
        nc.dram_tensor("b2", [D_MODEL], f32, kind="ExternalInput").ap(),
        nc.dram_tensor("consts", [4], f32, kind="ExternalInput").ap(),
        nc.dram_tensor("outT", [D_MODEL, Q], f32, kind="ExternalOutput").ap(),
    ]
    with tile.TileContext(nc) as tc:
        _emit(nc, tc, aps)
    nc.compile()
    _BUILT = nc
    return nc


def make_in_maps(inputs):
    import ml_dtypes

    bf16 = ml_dtypes.bfloat16
    e4 = ml_dtypes.float8_e4m3
    x = np.asarray(inputs["x"], np.float32)
    src_mask = np.asarray(inputs["src_mask"], np.int32)

    def w8(name):
        # [D_MODEL, out] -> [P, KD, out] with feature f = k*P + p
        w = np.asarray(inputs[name], np.float32) * WSC
        w = w.reshape(KD, P, -1).transpose(1, 0, 2)
        return np.ascontiguousarray(w.astype(e4)).view(np.uint8)

    shared = {
        "Wq": w8("Wq"), "Wk": w8("Wk"), "Wv": w8("Wv"), "Wo": w8("Wo"),
        "W1": np.ascontiguousarray(np.asarray(inputs["W1"], np.float32)
                                   .reshape(KD, P, D_FF).transpose(1, 0, 2).astype(bf16)),
        "W2": np.ascontiguousarray(np.asarray(inputs["W2"], np.float32)
                                   .reshape(FJ, P, D_MODEL).transpose(1, 0, 2).astype(bf16)),
        "bq": np.ascontiguousarray(np.asarray(inputs["bq"], np.float32)),
        "bk": np.ascontiguousarray(np.asarray(inputs["bk"], np.float32)),
        "bv": np.ascontiguousarray(np.asarray(inputs["bv"], np.float32)),
        "bo": np.ascontiguousarray(np.asarray(inputs["bo"], np.float32)),
        "b1": np.ascontiguousarray(np.asarray(inputs["b1"], np.float32)),
        "b2": np.ascontiguousarray(np.asarray(inputs["b2"], np.float32)),
        "consts": np.ascontiguousarray(
            np.array(
                [
                    np.asarray(inputs["alpha1"]).reshape(-1)[0] * np.sqrt(511.0),
                    np.asarray(inputs["beta1"]).reshape(-1)[0],
                    np.asarray(inputs["alpha2"]).reshape(-1)[0] * np.sqrt(511.0),
                    np.asarray(inputs["beta2"]).reshape(-1)[0],
                ],
                np.float32,
            )
        ),
    }
    in_maps = []
    for c in range(N_CORES):
        b = c // CORES_PER_BATCH
        qs = (c % CORES_PER_BATCH) * Q
        x_rot = np.concatenate([x[b, qs:, :], x[b, :qs, :]], axis=0)
        m_b = src_mask[b, 0, 0, :]
        m_rot = np.concatenate([m_b[qs:], m_b[:qs]], axis=0)
        in_map = dict(shared)
        # xT[p, t, k, tt] = x_rot[512t+tt, 128k+p]
        xr = x_rot.reshape(TC, 512, KD, P)
        in_map["xT"] = np.ascontiguousarray(
            xr.transpose(3, 0, 2, 1).astype(bf16))
        # xq[p, k, t] = x_rot[t, 128k+p]
        in_map["xq"] = np.ascontiguousarray(
            x_rot[0:Q, :].reshape(Q, KD, P).transpose(2, 1, 0))
        in_map["mask"] = np.ascontiguousarray(m_rot)
        in_maps.append(in_map)
    return in_maps


def assemble_output(results):
    out = np.empty((B, S, D_MODEL), np.float32)
    for c in range(N_CORES):
        b = c // CORES_PER_BATCH
        qs = (c % CORES_PER_BATCH) * Q
        out[b, qs:qs + Q, :] = results[c]["outT"].T
    return out


def kernel(**inputs):
    from concourse.bass_utils import run_bass_kernel_spmd

    nc = _build()
    in_maps = make_in_maps(inputs)
    res = run_bass_kernel_spmd(nc, in_maps, core_ids=list(range(N_CORES)))
    return assemble_output(res.results)
